# revision 1
# baseline (speedup 1.0000x reference)
"""Trainium2 Bass kernel for hierarchical 2-layer GAT (nn_GAT_20383914787079).

Data-parallel over 8 NeuronCores: each core owns B/8 = 128 root nodes and
their full neighbor subtree (1280 level-1 rows, 32000 level-2 rows).

Restructured GAT (mathematically identical to the reference):
  per head h:  agg_h = A_h @ x_neigh   (A_h = block-diag softmax alphas)
               g[:, h*D:(h+1)*D] = agg_h @ W_h
  attention logits via precombined vectors u = W@a_self, v = W@a_neigh:
               es = x_self @ u,  en = x_neigh @ v
So the expensive projection of all 32000 neighbor rows is replaced by
projecting the 1280 aggregated rows; the h2 stream feeds only (a) a
transpose+tiny-matmul for `en` and (b) the alpha-weighted aggregation.

All large tensors are bf16 on-chip (fp32 HBM reads, cast during DMA).
"""
import numpy as np
import ml_dtypes

import concourse.bass as bass
import concourse.tile as tile
from concourse import mybir
from concourse.vector_clock import ScopedClock
from concourse.bass_utils import run_bass_kernel_spmd

BF = mybir.dt.bfloat16
F32 = mybir.dt.float32
NPBF = ml_dtypes.bfloat16

NEG = 0.2
NCORES = 8
B, FD, D, H, OUT = 128, 128, 128, 2, 128   # per-core roots, dims
R0, R1 = 10, 25
M1 = B * R0                                 # 1280 level-1 rows per core
M2 = M1 * R1                                # 32000 level-2 rows per core

# tiling
T2, G2, NC2 = 100, 4, 320      # j1: k=25 tiles
T1, G1, NC1 = 80, 8, 16        # j0/L1: k=10 tiles
NBLK, CPB = 5, 64              # j1 processed in 5 blocks of 64 chunks


def _install_drain_patch():
    """This container's walrus rejects >1 sync-wait per instruction; split the
    Tile tail-drain waits across SP NoOps."""
    def _patched(self, tick_clock, wait_clock):
        nc = self.nc
        probe = nc.sync.nop(nofuse=True, hint="drain_wait_split")
        wait_clock.add_sem_waits(probe.ins,
                                 ScopedClock({None: tick_clock.global_clock}))
        si = probe.ins.sync_info
        waits = list(si.on_wait) if si is not None and si.on_wait else []
        if len(waits) > 1:
            si.on_wait = [waits[0]]
            for wi in waits[1:]:
                n2 = nc.sync.nop(nofuse=True, hint="drain_wait_split")
                if n2.ins.sync_info is None:
                    n2.ins.sync_info = mybir.SyncInfo(on_wait=[wi], on_update=[])
                else:
                    n2.ins.sync_info.on_wait = [wi]
        nc.sync.drain()
        nc.all_engine_barrier()
        popped = nc._tile_sem_poison_stack.pop()
        assert popped is self._sem_poison
        nc.clear_and_free_semaphores(list(self.sems.allocated().values()))
        nc.all_engine_barrier()

    tile.TileContext._drain_and_barrier = _patched


def _split_multi_waits(nc):
    """Walrus here allows only one sync-wait per instruction: hoist extra
    waits onto same-engine NoOps inserted immediately before."""
    nid = [0]
    for fn in nc.m.functions:
        for bb in fn.blocks:
            insts = bb.instructions
            i = 0
            while i < len(insts):
                inst = insts[i]
                si = inst.sync_info
                if si is not None and si.on_wait and len(si.on_wait) > 1:
                    waits = list(si.on_wait)
                    si.on_wait = [waits[-1]]
                    for w in waits[:-1]:
                        nid[0] += 1
                        nop = mybir.InstNoOp(
                            name=f"waitsplit-{nid[0]}", ins=[], outs=[],
                            sync_info=mybir.SyncInfo(on_wait=[w], on_update=[]))
                        nop.engine = inst.engine
                        insts.insert(i, nop)
                        i += 1
                i += 1


def host_params(W0, a_s0, a_n0, W1, a_s1, a_n1, fc_W):
    """Small parameter prep on host (fp64 for the tiny contractions).
    All params are packed into two [128, N] blobs (one bf16, one fp32) so the
    device needs just two parameter DMAs."""
    W0c = np.transpose(np.float64(W0), (1, 0, 2)).reshape(FD, H * D)
    W1c = np.transpose(np.float64(W1), (1, 0, 2)).reshape(H * D, H * D)
    u0 = np.einsum("hfd,hd->fh", np.float64(W0), np.float64(a_s0))
    v0 = np.einsum("hfd,hd->fh", np.float64(W0), np.float64(a_n0))
    u1 = np.einsum("hcd,hd->ch", np.float64(W1), np.float64(a_s1))
    v1 = np.einsum("hcd,hd->ch", np.float64(W1), np.float64(a_n1))
    fcW = np.float64(fc_W)
    eye = np.eye(128)
    masks = {}
    for K, T in ((25, T2), (10, T1)):
        G = T // K
        m = (np.arange(T)[:, None] // K == np.arange(G)[None, :]).astype(np.float64)
        mp = np.zeros((128, G)); mp[:T] = m
        mep = np.zeros((128, T)); mep[:G] = m.T
        masks[K] = (mp, mep)

    def pad(a):
        out = np.zeros((128, a.shape[1]))
        out[:a.shape[0]] = a
        return out

    bf_parts = [W0c, W1c[:128], W1c[128:],
                np.concatenate([u1[:128], u1[128:]], axis=1),
                np.concatenate([v1[:128], v1[128:]], axis=1),
                fcW[:128], fcW[128:], eye, pad(masks[10][0][:T1])]
    f32_parts = [eye, u0, v0, masks[25][0], masks[25][1], masks[25][0],
                 masks[10][0], masks[10][1], masks[10][0]]
    pbf = np.concatenate(bf_parts, axis=1).astype(NPBF)
    pf32 = np.concatenate(f32_parts, axis=1).astype(np.float32)
    return {"pbf": np.ascontiguousarray(pbf),
            "pf32": np.ascontiguousarray(pf32)}


# column offsets in the blobs
BF_COLS = {"W0c": (0, 256), "W1ca": (256, 512), "W1cb": (512, 768),
           "u1": (768, 772), "v1": (772, 776), "fcWa": (776, 904),
           "fcWb": (904, 1032), "identb": (1032, 1160), "stair10b": (1160, 1168)}
F32_COLS = {"identf": (0, 128), "u0": (128, 130), "v0": (130, 132),
            "mask25": (132, 136), "maskE25": (136, 236), "stair25f": (236, 240),
            "mask10": (240, 248), "maskE10": (248, 328), "stair10f": (328, 336)}
BF_N = 1168
F32_N = 336


def build_program(split_waits=True):
    nc = bass.Bass()
    dp = nc.declare_dram_parameter
    h0 = dp("h0", [B, FD], F32, isOutput=False)
    h1 = dp("h1", [M1, FD], F32, isOutput=False)
    h2 = dp("h2", [M2, FD], F32, isOutput=False)
    y = dp("y", [B, OUT], F32, isOutput=True)
    pbf_d = dp("pbf", [128, BF_N], BF, isOutput=False)
    pf32_d = dp("pf32", [128, F32_N], F32, isOutput=False)

    copy_ctr = [0]

    with tile.TileContext(nc) as tc:
        with (tc.tile_pool(name="prm", bufs=1) as prm,
              tc.tile_pool(name="big", bufs=1) as big,
              tc.tile_pool(name="h2p", bufs=3) as h2p,
              tc.tile_pool(name="x2t", bufs=3) as x2tp,
              tc.tile_pool(name="work", bufs=3) as wk,
              tc.tile_pool(name="pt", bufs=2, space="PSUM") as ppt,
              tc.tile_pool(name="pen", bufs=1, space="PSUM") as ppen,
              tc.tile_pool(name="pagg", bufs=2, space="PSUM") as ppagg,
              tc.tile_pool(name="psm", bufs=2, space="PSUM") as ppsm):

            def cp(dst, src):
                """PSUM->SBUF (or sbuf) copy, alternating DVE/ACT."""
                copy_ctr[0] += 1
                if copy_ctr[0] % 2:
                    nc.vector.tensor_copy(dst, src)
                else:
                    nc.scalar.copy(dst, src)

            # ---- params to SBUF (two blob DMAs) ----
            pbf = prm.tile([128, BF_N], BF, tag="pbf")
            nc.sync.dma_start(pbf[:], pbf_d[:])
            pf32 = prm.tile([128, F32_N], F32, tag="pf32")
            nc.sync.dma_start(pf32[:], pf32_d[:])
            S = {}
            for nm, (c0, c1) in BF_COLS.items():
                S[nm] = pbf[:, c0:c1]
            for nm, (c0, c1) in F32_COLS.items():
                S[nm] = pf32[:, c0:c1]
            identb, identf = S["identb"], S["identf"]
            mask25 = S["mask25"][:T2, :]
            maskE25 = S["maskE25"][:G2, :]
            stair25 = S["stair25f"][:T2, :]
            mask10 = S["mask10"][:T1, :]
            maskE10 = S["maskE10"][:G1, :]
            stair10f = S["stair10f"][:T1, :]
            stair10b = S["stair10b"][:T1, :]

            # ---- h2 block 0 first (keeps the DMA device busy from t=0),
            # then h1/h0, then the remaining h2 blocks ----
            BLOCKS = [64, 64, 64, 48, 40, 24, 16]   # sums to NC2; 16-aligned
            hbs, c0s = [], []
            h2v = h2.rearrange("(c p) f -> p c f", p=T2)
            _off = [0]

            def issue_hb():
                b = len(hbs)
                bn, c0 = BLOCKS[b], _off[0]
                hb = h2p.tile([T2, bn * FD], F32, tag="h2blk", name="hb")
                nc.sync.dma_start(
                    hb[:].rearrange("p (c f) -> p c f", f=FD),
                    h2v[:, c0:c0 + bn, :])
                hbs.append(hb)
                c0s.append(c0)
                _off[0] += bn

            # ---- h0 / h1 loads (fp32, HWDGE) ----
            h1row = big.tile([T1, NC1 * FD], F32, tag="h1row")     # [80, 2048]
            nc.sync.dma_start(
                h1row[:].rearrange("p (c f) -> p c f", f=FD),
                h1.rearrange("(c p) f -> p c f", p=T1))
            h0row = big.tile([128, FD], F32, tag="h0row")
            nc.sync.dma_start(h0row[:], h0[:])
            for _ in range(len(BLOCKS)):
                issue_hb()

            # ---- X1T: transpose h1 tiles -> [128, 1280] fp32 ----
            x1t = big.tile([128, M1], F32, tag="x1t")
            for grp in range(4):      # 4 groups of 4 tiles -> psum [128, 320]
                pt = ppt.tile([128, 4 * T1], F32, tag="pt")
                for j in range(4):
                    cidx = 4 * grp + j
                    nc.tensor.transpose(pt[:, j * T1:(j + 1) * T1],
                                        h1row[:, cidx * FD:(cidx + 1) * FD],
                                        identf[:T1, :T1])
                cp(x1t[:, grp * 4 * T1:(grp + 1) * 4 * T1], pt[:])

            # h0T
            pt = ppt.tile([128, 128], F32, tag="pt")
            nc.tensor.transpose(pt[:], h0row[:], identf[:])
            h0t = big.tile([128, 128], F32, tag="h0t")
            cp(h0t[:], pt[:])

            # ---- es1 (j1 self): lhsT=u0, rhs=X1T strips -> es1T [2, 1280] ----
            es1T = big.tile([2, M1], F32, tag="es1T")
            for w, (c0, c1) in enumerate(((0, 6), (6, 12), (12, 16))):
                psw = ppsm.tile([2, 480], F32, tag="psm")
                for cc in range(c0, c1):
                    nc.tensor.matmul(psw[:, (cc - c0) * T1:(cc - c0 + 1) * T1],
                                     S["u0"][:], x1t[:, cc * T1:(cc + 1) * T1])
                cp(es1T[:, c0 * T1:c1 * T1], psw[:, :(c1 - c0) * T1])
            # es1_arr[g', NC2*h + c] = es1[m=4c+g', h]
            es1_arr = big.tile([G2, 2 * NC2], F32, tag="es1_arr")
            for g in range(G2):
                for h in range(H):
                    nc.gpsimd.dma_start(
                        es1_arr[g:g + 1, NC2 * h:NC2 * (h + 1)],
                        es1T[h:h + 1, :].rearrange("o (c g) -> o c g", g=G2)[:, :, g])

            # ---- es0 (j0 self): lhsT=u0, rhs=h0T -> es0T [2, 128] ----
            ps0 = ppsm.tile([2, 128], F32, tag="psm")
            nc.tensor.matmul(ps0[:], S["u0"][:], h0t[:])
            es0T = big.tile([2, 128], F32, tag="es0T")
            cp(es0T[:], ps0[:])
            es0_arr = big.tile([G1, 2 * NC1], F32, tag="es0_arr")
            for g in range(G1):
                for h in range(H):
                    nc.gpsimd.dma_start(
                        es0_arr[g:g + 1, NC1 * h:NC1 * (h + 1)],
                        es0T[h:h + 1, :].rearrange("o (c g) -> o c g", g=G1)[:, :, g])

            # ---- en1 (j0 neigh): lhsT=X1T tile, rhs=v0 -> [80, 2] windows ----
            pen1 = ppen.tile([T1, 2 * NC1], F32, tag="pen")
            for cc in range(NC1):
                nc.tensor.matmul(pen1[:, 2 * cc:2 * cc + 2],
                                 x1t[:, cc * T1:(cc + 1) * T1], S["v0"][:])
            en1 = big.tile([T1, 2 * NC1], F32, tag="en1")
            cp(en1[:], pen1[:])

            # ============ shared softmax+agg machinery ============
            def softmax_alpha(T, G, K, NCc, en_sb, es_arr_slice, mask, maskE,
                              stair, adt, blk_tag):
                """en_sb [T, 2*NCc] fp32 sbuf; es_arr_slice [G, NCc, H] fp32 AP.
                Returns astrip [T, NCc*2*G] in dtype adt."""
                W = 2 * NCc
                pE = ppsm.tile([T, W], F32, tag="psm")          # esE expand
                nc.tensor.matmul(pE[:], maskE[:], es_arr_slice)
                e1 = wk.tile([T, W], F32, tag=f"e1{blk_tag}")
                nc.vector.tensor_add(e1[:], en_sb, pE[:])
                e2 = wk.tile([T, W], F32, tag=f"e2{blk_tag}")
                nc.vector.scalar_tensor_tensor(e2[:], e1[:], NEG, e1[:],
                                               mybir.AluOpType.mult,
                                               mybir.AluOpType.max)
                ex = wk.tile([T, W], F32, tag=f"ex{blk_tag}")
                nc.scalar.activation(ex[:], e2[:], mybir.ActivationFunctionType.Exp)
                pden = ppsm.tile([G, W], F32, tag="psm")
                nc.tensor.matmul(pden[:], mask[:], ex[:])
                den = wk.tile([G, W], F32, tag=f"den{blk_tag}")
                cp(den[:], pden[:])
                rden = wk.tile([G, W], F32, tag=f"rden{blk_tag}")
                nc.vector.reciprocal(rden[:], den[:])
                pdE = ppsm.tile([T, W], F32, tag="psm")
                nc.tensor.matmul(pdE[:], maskE[:], rden[:])
                alpha = wk.tile([T, W], adt, tag=f"al{blk_tag}")
                nc.vector.tensor_mul(alpha[:], ex[:], pdE[:])
                astrip = wk.tile([T, NCc * 2 * G], adt, tag=f"as{blk_tag}")
                a4 = alpha[:].rearrange("p (c h) -> p c h", h=H)
                a4 = a4.unsqueeze(3).broadcast_to([T, NCc, H, G])
                s4 = stair.unsqueeze(1).unsqueeze(1).broadcast_to([T, NCc, H, G])
                nc.vector.tensor_mul(
                    astrip[:].rearrange("p (c h g) -> p c h g", h=H, g=G), a4, s4)
                return astrip

            # ============ j0: agg over h1 (output transposed: [f, m-cols]) ====
            es0_v = es0_arr[:].rearrange("g (h c) -> g c h", h=H)
            astrip0 = softmax_alpha(T1, G1, R0, NC1, en1[:], es0_v,
                                    mask10, maskE10, stair10f, F32, "j0")
            # aggT0[f, 16*cc + 8h + g] = sum_p h1row[p, cc*FD+f]*astrip0[p, ...]
            pgj0 = ppagg.tile([128, 2 * G1 * NC1], F32, tag="paggs", bufs=1)
            for cc in range(NC1):
                nc.tensor.matmul(pgj0[:, 16 * cc:16 * cc + 16],
                                 h1row[:, cc * FD:(cc + 1) * FD],
                                 astrip0[:, 16 * cc:16 * cc + 16])
            aggT0 = big.tile([128, 2 * G1 * NC1], BF, tag="aggT0")
            cp(aggT0[:], pgj0[:])

            # j0 projection -> g0t [128 d, (h, m0)], m0 = 8cc + g
            g0t = big.tile([128, 2 * B], BF, tag="g0t")
            pj0 = ppagg.tile([128, 2 * B], F32, tag="paggs", bufs=1)
            for h in range(H):
                rhs = aggT0[:].rearrange("p (cc h g) -> p cc h g",
                                         h=H, g=G1)[:, :, h, :]
                nc.tensor.matmul(pj0[:, B * h:B * (h + 1)],
                                 S["W0c"][:, 128 * h:128 * (h + 1)], rhs)
            cp(g0t[:], pj0[:])

            # es_L1: lhsT = u1 chunks, rhs = g0T slabs -> [2, 128] accumulate
            psL = ppsm.tile([2, B], F32, tag="psm")
            for hp in range(H):
                nc.tensor.matmul(psL[:], S["u1"][:, 2 * hp:2 * hp + 2],
                                 g0t[:, B * hp:B * (hp + 1)],
                                 start=(hp == 0), stop=(hp == 1))
            esLT = big.tile([2, B], F32, tag="esLT")
            cp(esLT[:], psL[:])
            esL_arr = big.tile([G1, 2 * NC1], F32, tag="esL_arr")
            for g in range(G1):
                for h in range(H):
                    nc.gpsimd.dma_start(
                        esL_arr[g:g + 1, NC1 * h:NC1 * (h + 1)],
                        esLT[h:h + 1, :].rearrange("o (c g) -> o c g", g=G1)[:, :, g])

            # ============ j1: stream h2 in NBLK blocks (fp32 tiles) ============
            en_sb = big.tile([T2, 2 * NC2], F32, tag="en_sb")      # [100, 640]
            # aggT1 [128 f, (q, cl, h, g)]: col = 128q + 8cl + 4h + g
            aggT1 = big.tile([128, 20 * 128], BF, tag="aggT1")
            g1t = big.tile([128, 2 * M1], BF, tag="g1t")
            g1row = big.tile([T1, NC1 * 2 * FD], BF, tag="g1row")
            pagg_cur = [None]
            g1row_done = [0]

            def project_q(q):
                # pj [128 d, (h, mloc)] for m-window [64q, 64q+64)
                pj = ppagg.tile([128, 128], F32, tag="paggs", bufs=1,
                                name="pj_q")
                for h in range(H):
                    rhs = aggT1[:, q * 128:(q + 1) * 128].rearrange(
                        "p (cl h g) -> p cl h g", h=H, g=G2)[:, :, h, :]
                    nc.tensor.matmul(pj[:, 64 * h:64 * h + 64],
                                     S["W0c"][:, 128 * h:128 * (h + 1)], rhs)
                dst = g1t[:].rearrange("p (h q m) -> p h q m",
                                       h=H, q=20)[:, :, q, :]
                cp(dst, pj[:].rearrange("p (h m) -> p h m", h=H))
                # g1row transposes for completed 80-wide windows
                while 80 * (g1row_done[0] + 1) <= 64 * (q + 1):
                    t = g1row_done[0]
                    for hp in range(H):
                        pt5 = ppt.tile([T1, 128], BF, tag="pt", name="pt5")
                        nc.tensor.transpose(
                            pt5[:],
                            g1t[:, M1 * hp + T1 * t: M1 * hp + T1 * (t + 1)],
                            identb[:])
                        cp(g1row[:, (2 * t + hp) * FD:(2 * t + hp + 1) * FD],
                           pt5[:])
                    g1row_done[0] += 1

            def agg1_flush(q):
                cp(aggT1[:, q * 128:(q + 1) * 128], pagg_cur[0][:])
                pagg_cur[0] = None
                project_q(q)

            def stage1(b):
                # transposes (pack 4 per psum tile) + en-mms
                hb, bn, c0 = hbs[b], BLOCKS[b], c0s[b]
                penb = ppen.tile([T2, 2 * bn], F32, tag="pen", name="penb")
                for grp in range((bn + 3) // 4):
                    cls = list(range(4 * grp, min(4 * grp + 4, bn)))
                    pt3 = ppt.tile([128, len(cls) * T2], F32, tag="pt",
                                   name="pt3")
                    for j, cl in enumerate(cls):
                        nc.tensor.transpose(pt3[:, j * T2:(j + 1) * T2],
                                            hb[:, cl * FD:(cl + 1) * FD],
                                            identf[:T2, :T2])
                    xs = x2tp.tile([128, len(cls) * T2], F32, tag="x2t",
                                   name="xs")
                    cp(xs[:], pt3[:])
                    for j, cl in enumerate(cls):
                        nc.tensor.matmul(penb[:, 2 * cl:2 * cl + 2],
                                         xs[:, j * T2:(j + 1) * T2], S["v0"][:])
                cp(en_sb[:, 2 * c0:2 * (c0 + bn)], penb[:])

            def stage2(b):
                hb, bn, c0 = hbs[b], BLOCKS[b], c0s[b]
                es1_v = es1_arr[:].rearrange("g (h c) -> g c h", h=H)[
                    :, c0:c0 + bn, :]
                astr = softmax_alpha(T2, G2, R1, bn,
                                     en_sb[:, 2 * c0:2 * (c0 + bn)],
                                     es1_v, mask25, maskE25, stair25, F32, "j1")
                for cl in range(bn):
                    ci = c0 + cl
                    q, r = divmod(ci, 16)
                    if r == 0:
                        pagg_cur[0] = ppagg.tile([128, 128], F32, tag="pagg",
                                                 name="pagg_j1")
                    nc.tensor.matmul(pagg_cur[0][:, 8 * r:8 * r + 8],
                                     hb[:, cl * FD:(cl + 1) * FD],
                                     astr[:, 8 * cl:8 * cl + 8])
                    if r == 15:
                        agg1_flush(q)

            # 2-stage software pipeline: engines run in program order, so a
            # block's softmax chain must not sit ahead of the next block's
            # independent transposes in any engine queue.
            for b in range(len(BLOCKS)):
                stage1(b)
                if b > 0:
                    stage2(b - 1)
            stage2(len(BLOCKS) - 1)

            # ============ L1 ============
            # en_L1: lhsT = g1T slices, rhs = v1 chunk, accumulate chunks
            penL = ppen.tile([T1, 2 * NC1], F32, tag="pen")
            for t in range(NC1):
                for hp in range(H):
                    nc.tensor.matmul(penL[:, 2 * t:2 * t + 2],
                                     g1t[:, M1 * hp + T1 * t: M1 * hp + T1 * (t + 1)],
                                     S["v1"][:, 2 * hp:2 * hp + 2],
                                     start=(hp == 0), stop=(hp == 1))
            enL = big.tile([T1, 2 * NC1], F32, tag="enL")
            cp(enL[:], penL[:])

            esL_v = esL_arr[:].rearrange("g (h c) -> g c h", h=H)
            astrL = softmax_alpha(T1, G1, R0, NC1, enL[:], esL_v,
                                  mask10, maskE10, stair10b, BF, "L1")
            # aggT2 [128 (fp-slab d), (fp, t, h, g)]: col = 256fp + 16t + 8h + g
            aggT2 = big.tile([128, 2 * 256], BF, tag="aggT2")
            for fp in range(2):
                pg = ppagg.tile([128, 256], F32, tag="paggs", bufs=1)
                for t in range(NC1):
                    nc.tensor.matmul(pg[:, 16 * t:16 * t + 16],
                                     g1row[:, (2 * t + fp) * FD:(2 * t + fp + 1) * FD],
                                     astrL[:, 16 * t:16 * t + 16])
                cp(aggT2[:, 256 * fp:256 * (fp + 1)], pg[:])

            # L1 projection: ggt [128 d, (h, m0)], m0 = 8t + g
            ggt = big.tile([128, 2 * B], BF, tag="ggt")
            W1cs = (S["W1ca"], S["W1cb"])
            pjL = ppagg.tile([128, 2 * B], F32, tag="paggs", bufs=1)
            for h in range(H):
                for fp in range(2):
                    rhs = aggT2[:, 256 * fp:256 * (fp + 1)].rearrange(
                        "p (t h g) -> p t h g", h=H, g=G1)[:, :, h, :]
                    nc.tensor.matmul(pjL[:, B * h:B * (h + 1)],
                                     W1cs[fp][:, 128 * h:128 * (h + 1)], rhs,
                                     start=(fp == 0), stop=(fp == 1))
            cp(ggt[:], pjL[:])

            # fc: outT [o, m0] = sum_chunks fcW_chunk.T @ ggT_slab
            pfc = ppagg.tile([128, B], F32, tag="paggs", bufs=1)
            fcs = (S["fcWa"], S["fcWb"])
            for hp in range(H):
                nc.tensor.matmul(pfc[:], fcs[hp][:], ggt[:, B * hp:B * (hp + 1)],
                                 start=(hp == 0), stop=(hp == 1))
            outT = big.tile([128, B], F32, tag="outT")
            cp(outT[:], pfc[:])
            ptf = ppt.tile([128, B], F32, tag="pt")
            nc.tensor.transpose(ptf[:], outT[:], identf[:])
            outn = big.tile([B, OUT], F32, tag="outn")
            cp(outn[:], ptf[:])
            nc.sync.dma_start(y[:], outn[:])

    if split_waits:
        _split_multi_waits(nc)
    return nc


_PROG = None


def kernel(**inputs):
    global _PROG
    _install_drain_patch()
    P = host_params(inputs["W0"], inputs["a_self0"], inputs["a_neigh0"],
                    inputs["W1"], inputs["a_self1"], inputs["a_neigh1"],
                    inputs["fc_W"])
    if _PROG is None:
        _PROG = build_program()
    nc = _PROG
    h0 = np.ascontiguousarray(np.asarray(inputs["h0"], np.float32))
    h1 = np.ascontiguousarray(np.asarray(inputs["h1"], np.float32))
    h2 = np.ascontiguousarray(np.asarray(inputs["h2"], np.float32))
    in_maps = []
    for c in range(NCORES):
        m = {"h0": h0[B * c:B * (c + 1)],
             "h1": h1[M1 * c:M1 * (c + 1)],
             "h2": h2[M2 * c:M2 * (c + 1)]}
        m.update(P)
        in_maps.append(m)
    core_ids = list(range(NCORES))
    last = None
    for _attempt in range(3):
        try:
            res = run_bass_kernel_spmd(nc, in_maps, core_ids)
            out = np.concatenate([np.asarray(res.results[c]["y"])
                                  for c in core_ids], axis=0)
            return out.astype(np.float32)
        except Exception as e:   # transient device-unrecoverable happens
            last = e
    raise last



# revision 14
# speedup vs baseline: 1.5596x; 1.5596x over previous
"""Trainium2 Bass kernel for hierarchical 2-layer GAT (nn_GAT_20383914787079).

Data-parallel over 8 NeuronCores: each core owns B/8 = 128 root nodes and
their full neighbor subtree (1280 level-1 rows, 32000 level-2 rows).

Restructured GAT (mathematically identical to the reference):
  per head h:  agg_h = A_h @ x_neigh   (A_h = block-diag softmax alphas)
               g[:, h*D:(h+1)*D] = agg_h @ W_h
  attention logits via precombined vectors u = W@a_self, v = W@a_neigh:
               es = x_self @ u,  en = x_neigh @ v

All hierarchy inputs are cast to bf16 and re-tiled on host so each DMA is
fully contiguous on both sides.  h2 is processed in 125-row chunks (5 softmax
groups of 25), streamed in blocks split across the SP and ACT DMA queues.
Self-logit broadcast into the (group, chunk) grid is done with tiny
mask-row matmuls on the tensor engine (no rearrange DMAs).
"""
import numpy as np
import ml_dtypes

import concourse.bass as bass
import concourse.tile as tile
from concourse import mybir
from concourse.vector_clock import ScopedClock
from concourse.bass_utils import run_bass_kernel_spmd

BF = mybir.dt.bfloat16
F32 = mybir.dt.float32
NPBF = ml_dtypes.bfloat16

NEG = 0.2
NCORES = 8
B, FD, D, H, OUT = 128, 128, 128, 2, 128   # per-core roots, dims
R0, R1 = 10, 25
M1 = B * R0                                 # 1280 level-1 rows per core
M2 = M1 * R1                                # 32000 level-2 rows per core

# j1 tiling: chunks of 125 rows = 5 groups of 25
T2, G2, NC2 = 125, 5, 256
# j0 / L1 tiling: chunks of 80 rows = 8 groups of 10
T1, G1, NC1 = 80, 8, 16

# j1 stream blocks: (chunk_count, dma_engine). Sizes multiple of 16 so the
# 80-row g1row windows close exactly at block boundaries.
BLOCKS = [(16, "scalar"), (32, "gpsimd"), (32, "sync"), (32, "gpsimd"),
          (32, "sync"), (32, "gpsimd"), (32, "sync"), (32, "gpsimd"),
          (16, "sync")]
assert sum(bn for bn, _ in BLOCKS) == NC2


def _install_drain_patch():
    """This container's walrus rejects >1 sync-wait per instruction; split the
    Tile tail-drain waits across SP NoOps."""
    def _patched(self, tick_clock, wait_clock):
        nc = self.nc
        probe = nc.sync.nop(nofuse=True, hint="drain_wait_split")
        wait_clock.add_sem_waits(probe.ins,
                                 ScopedClock({None: tick_clock.global_clock}))
        si = probe.ins.sync_info
        waits = list(si.on_wait) if si is not None and si.on_wait else []
        if len(waits) > 1:
            si.on_wait = [waits[0]]
            for wi in waits[1:]:
                n2 = nc.sync.nop(nofuse=True, hint="drain_wait_split")
                if n2.ins.sync_info is None:
                    n2.ins.sync_info = mybir.SyncInfo(on_wait=[wi], on_update=[])
                else:
                    n2.ins.sync_info.on_wait = [wi]
        nc.sync.drain()
        nc.all_engine_barrier()
        popped = nc._tile_sem_poison_stack.pop()
        assert popped is self._sem_poison
        nc.clear_and_free_semaphores(list(self.sems.allocated().values()))
        nc.all_engine_barrier()

    tile.TileContext._drain_and_barrier = _patched


def _split_multi_waits(nc):
    """Walrus here allows only one sync-wait per instruction: hoist extra
    waits onto same-engine NoOps inserted immediately before."""
    nid = [0]
    for fn in nc.m.functions:
        for bb in fn.blocks:
            insts = bb.instructions
            i = 0
            while i < len(insts):
                inst = insts[i]
                si = inst.sync_info
                if si is not None and si.on_wait and len(si.on_wait) > 1:
                    waits = list(si.on_wait)
                    si.on_wait = [waits[-1]]
                    for w in waits[:-1]:
                        nid[0] += 1
                        nop = mybir.InstNoOp(
                            name=f"waitsplit-{nid[0]}", ins=[], outs=[],
                            sync_info=mybir.SyncInfo(on_wait=[w], on_update=[]))
                        nop.engine = inst.engine
                        insts.insert(i, nop)
                        i += 1
                i += 1


# ---------------- host-side parameter / input prep ----------------

# bf16 blob column layout
BF_COLS = {}
_off = [0]


def _col(name, w):
    BF_COLS[name] = (_off[0], _off[0] + w)
    _off[0] += w


_col("identb", 128)
_col("mask25", G2)              # [125, 5]
_col("maskE25", G2 * T2)        # 5 rows of [1, 125] side by side
_col("u0", H)
_col("v0", H)
_col("W0c", H * D)
_col("W1ca", H * D)
_col("W1cb", H * D)
_col("u1", H * H)               # [128, (hp, h)]
_col("v1", H * H)
_col("fcWa", OUT)
_col("fcWb", OUT)
_col("zrow", 64)                # zeros (psum accumulation opener)
_col("mask10", G1)              # [80, 8]
_col("maskE10", G1 * T1)        # 8 rows of [1, 80]
BF_N = _off[0]

# maskE25f stored [5 rows, 125]; maskE10f stored [8 rows, 80]
F32_COLS = {"maskE25f": (0, 125), "maskE10f": (125, 205)}
F32_N = 205


def host_params(W0, a_s0, a_n0, W1, a_s1, a_n1, fc_W):
    W0c = np.transpose(np.float64(W0), (1, 0, 2)).reshape(FD, H * D)
    W1c = np.transpose(np.float64(W1), (1, 0, 2)).reshape(H * D, H * D)
    u0 = np.einsum("hfd,hd->fh", np.float64(W0), np.float64(a_s0))
    v0 = np.einsum("hfd,hd->fh", np.float64(W0), np.float64(a_n0))
    u1 = np.einsum("hcd,hd->ch", np.float64(W1), np.float64(a_s1))
    v1 = np.einsum("hcd,hd->ch", np.float64(W1), np.float64(a_n1))
    fcW = np.float64(fc_W)

    def mk_masks(T, G, K):
        m = (np.arange(T)[:, None] // K == np.arange(G)[None, :]).astype(
            np.float64)                       # [T, G]
        return m

    m25 = mk_masks(T2, G2, R1)
    m10 = mk_masks(T1, G1, R0)

    pbf = np.zeros((128, BF_N))
    pf32 = np.zeros((128, F32_N))

    def put(dst, name, arr):
        c0, c1 = (BF_COLS if dst is pbf else F32_COLS)[name]
        assert c1 - c0 == arr.shape[1], name
        dst[:arr.shape[0], c0:c1] = arr

    put(pbf, "identb", np.eye(128))
    put(pbf, "mask25", m25)
    put(pbf, "maskE25", m25.T.reshape(1, -1))     # row0 = [maskE_g rows concat]
    put(pbf, "u0", u0)
    put(pbf, "v0", v0)
    put(pbf, "W0c", W0c)
    put(pbf, "W1ca", W1c[:128])
    put(pbf, "W1cb", W1c[128:])
    put(pbf, "u1", np.concatenate([u1[:128], u1[128:]], axis=1))
    put(pbf, "v1", np.concatenate([v1[:128], v1[128:]], axis=1))
    put(pbf, "fcWa", fcW[:128])
    put(pbf, "fcWb", fcW[128:])
    put(pbf, "mask10", m10)
    put(pbf, "maskE10", m10.T.reshape(1, -1))
    put(pf32, "maskE25f", m25.T)                  # [5, 125]
    put(pf32, "maskE10f", m10.T)                  # [8, 80]

    return {"pbf": np.ascontiguousarray(pbf.astype(NPBF)),
            "pf32": np.ascontiguousarray(pf32.astype(np.float32))}


def pack_h(h, T):
    """[N, F] fp32 -> [T, (N/T)*F] bf16, chunk-major contiguous."""
    n = h.shape[0] // T
    out = h.reshape(n, T, FD).transpose(1, 0, 2).reshape(T, n * FD)
    return np.ascontiguousarray(out.astype(NPBF))


# ---------------- device program ----------------

def build_program(split_waits=True):
    nc = bass.Bass()
    dp = nc.declare_dram_parameter
    h0d = dp("h0p", [B, FD], BF, isOutput=False)
    h1d = dp("h1p", [T1, NC1 * FD], BF, isOutput=False)
    h2d = dp("h2p", [T2, NC2 * FD], BF, isOutput=False)
    y = dp("y", [B, OUT], F32, isOutput=True)
    pbf_d = dp("pbf", [128, BF_N], BF, isOutput=False)
    pf32_d = dp("pf32", [128, F32_N], F32, isOutput=False)

    cp_ctr = [0]

    with tile.TileContext(nc) as tc:
        with (tc.tile_pool(name="prm", bufs=1) as prm,
              tc.tile_pool(name="big", bufs=1) as big,
              tc.tile_pool(name="h2p", bufs=4) as h2p,
              tc.tile_pool(name="x2t", bufs=4) as x2tp,
              tc.tile_pool(name="work", bufs=3) as wk,
              tc.tile_pool(name="agt", bufs=2) as agt,
              tc.tile_pool(name="pt", bufs=2, space="PSUM") as ppt,
              tc.tile_pool(name="pen", bufs=2, space="PSUM") as ppen,
              tc.tile_pool(name="penL", bufs=1, space="PSUM") as ppenL,
              tc.tile_pool(name="pagg", bufs=2, space="PSUM") as ppagg,
              tc.tile_pool(name="psm", bufs=1, space="PSUM") as ppsm):

            def pt_tile(name):
                # 126-col slab stride keeps bf16 PSUM writes 4B-aligned
                return ppt.tile([128, 504], BF, tag="pt", name=name)

            def cp(dst, src, eng=None):
                """PSUM->SBUF copy on a chosen/rotating engine."""
                if eng is None:
                    cp_ctr[0] += 1
                    eng = ("vector", "scalar")[cp_ctr[0] % 2]
                if eng == "scalar":
                    nc.scalar.copy(dst, src)
                elif eng == "gpsimd":
                    nc.gpsimd.tensor_copy(dst, src)
                else:
                    nc.vector.tensor_copy(dst, src)

            # ---- params ----
            pbf = prm.tile([128, BF_N], BF, tag="pbf")
            nc.sync.dma_start(pbf[:], pbf_d[:])
            S = {nm: pbf[:, c0:c1] for nm, (c0, c1) in BF_COLS.items()}
            identb = S["identb"]
            mask25 = S["mask25"][:T2, :]
            mask10 = S["mask10"][:T1, :]

            def maskE25b(g):
                c0, _ = BF_COLS["maskE25"]
                return pbf[0:1, c0 + T2 * g: c0 + T2 * (g + 1)]

            def maskE10b(g):
                c0, _ = BF_COLS["maskE10"]
                return pbf[0:1, c0 + T1 * g: c0 + T1 * (g + 1)]

            # ---- first h2 block on ACT queue before anything else ----
            hbs, c0s = [], []
            _off = [0]

            def issue_hb(i):
                bn, eng = BLOCKS[i]
                c0 = _off[0]
                hb = h2p.tile([T2, bn * FD], BF, tag="h2blk", name="hb")
                getattr(nc, {"sync": "sync", "scalar": "scalar",
                             "gpsimd": "gpsimd"}[eng]).dma_start(
                    hb[:], h2d[:, c0 * FD:(c0 + bn) * FD])
                hbs.append(hb)
                c0s.append(c0)
                _off[0] += bn

            issue_hb(0)

            # ---- h1 / h0 / f32 params on SP ----
            h1row = big.tile([T1, NC1 * FD], BF, tag="h1row")
            nc.sync.dma_start(h1row[:], h1d[:])
            h0row = big.tile([B, FD], BF, tag="h0row")
            nc.sync.dma_start(h0row[:], h0d[:])
            pf32 = prm.tile([128, F32_N], F32, tag="pf32")
            nc.sync.dma_start(pf32[:], pf32_d[:])
            maskE25f = pf32[:G2, 0:125]
            maskE10f = pf32[:G1, 125:205]

            issue_hb(1)
            issue_hb(2)

            # ================= shared softmax machinery ==================
            def softmax_alpha(T, G, W, pen, mask_b, maskEf, tagp):
                """pen: PSUM [T, W] logits (cols h-major: h*Wh + c).
                Returns astr sbuf bf16 [T, W*G] (cols (h, c, g))."""
                # exp(leaky_relu(x)) == max(exp(x), exp(NEG*x))
                ex1 = wk.tile([T, W], BF, tag=f"e1{tagp}")
                nc.scalar.activation(ex1[:], pen,
                                     mybir.ActivationFunctionType.Exp)
                ex2 = wk.tile([T, W], BF, tag=f"e2{tagp}")
                nc.scalar.activation(ex2[:], pen,
                                     mybir.ActivationFunctionType.Exp,
                                     scale=NEG)
                ex = wk.tile([T, W], BF, tag=f"ex{tagp}")
                nc.vector.scalar_tensor_tensor(ex[:], ex1[:], 1.0, ex2[:],
                                               mybir.AluOpType.mult,
                                               mybir.AluOpType.max)
                pden = ppsm.tile([G, W], F32, tag="psm", name="pden")
                nc.tensor.matmul(pden[:], mask_b, ex[:])
                rden = wk.tile([G, W], F32, tag=f"rd{tagp}")
                nc.vector.reciprocal(rden[:], pden[:])
                ppde = ppsm.tile([T, W], F32, tag="psm", name="ppde")
                nc.tensor.matmul(ppde[:], maskEf, rden[:])
                alpha = wk.tile([T, W], BF, tag=f"al{tagp}")
                nc.vector.tensor_mul(alpha[:], ex[:], ppde[:])
                astr = wk.tile([T, W * G], BF, tag=f"as{tagp}")
                a4 = alpha[:].unsqueeze(2).broadcast_to([T, W, G])
                s4 = mask_b.unsqueeze(1).broadcast_to([T, W, G])
                nc.vector.tensor_mul(
                    astr[:].rearrange("p (w g) -> p w g", g=G), a4, s4)
                return astr

            # ================= j0 (h0 <- h1) =================
            es1h, es0h, esLh = [], [], []

            def j0a():
                # x1t transposes + j1 self-logits (needed by stage2(0))
                for q in range(4):
                    pt = pt_tile("ptx1")
                    for j in range(4):
                        cidx = 4 * q + j
                        nc.tensor.transpose(pt[:, j * T1:(j + 1) * T1],
                                            h1row[:, cidx * FD:(cidx + 1) * FD],
                                            identb[:T1, :T1])
                    cp(x1t[:, q * 4 * T1:(q + 1) * 4 * T1], pt[:, :4 * T1],
                       "vector")
                for h in range(H):
                    eh = big.tile([1, M1], BF, tag=f"es1h{h}")
                    for (p0, p1) in ((0, 480), (480, 960), (960, 1280)):
                        pes = ppagg.tile([128, 480], F32, tag="pagg",
                                         name="pes1")
                        nc.tensor.matmul(pes[:1, :p1 - p0],
                                         S["u0"][:, h:h + 1], x1t[:, p0:p1])
                        cp(eh[:, p0:p1], pes[:1, :p1 - p0], "scalar")
                    es1h.append(eh)

            def j0b():
                # rest of level-0 layer (feeds only the final L1 stage)
                pt0 = pt_tile("pth0")
                nc.tensor.transpose(pt0[:, :128], h0row[:], identb[:])
                h0t = big.tile([128, 128], BF, tag="h0t")
                cp(h0t[:], pt0[:, :128], "vector")
                for h in range(H):
                    pes = ppagg.tile([128, 480], F32, tag="pagg", name="pes0")
                    nc.tensor.matmul(pes[:1, :B], S["u0"][:, h:h + 1], h0t[:])
                    eh = big.tile([1, B], BF, tag=f"es0h{h}")
                    cp(eh[:], pes[:1, :B], "scalar")
                    es0h.append(eh)
                pen0 = ppsm.tile([T1, 2 * NC1], F32, tag="psm", name="pen0")
                nc.tensor.matmul(pen0[:], maskE10b(0),
                                 S["zrow"][0:1, :2 * NC1],
                                 start=True, stop=False, skip_group_check=True)
                for cc in range(NC1):
                    for h in range(H):
                        nc.tensor.matmul(
                            pen0[:, NC1 * h + cc: NC1 * h + cc + 1],
                            x1t[:, cc * T1:(cc + 1) * T1],
                            S["v0"][:, h:h + 1],
                            start=False, stop=False, skip_group_check=True)
                for g in range(G1):
                    for h in range(H):
                        nc.tensor.matmul(
                            pen0[:, NC1 * h:NC1 * (h + 1)],
                            maskE10b(g),
                            es0h[h][:].rearrange("o (c g) -> o c g",
                                                 g=G1)[:, :, g],
                            start=False, stop=(g == G1 - 1 and h == H - 1),
                            skip_group_check=True)
                astr0 = softmax_alpha(T1, G1, 2 * NC1, pen0[:], mask10,
                                      maskE10f, "j0")
                pg0 = ppagg.tile([128, 2 * G1 * NC1], F32, tag="pagg",
                                 name="pg0")
                for cc in range(NC1):
                    rhs = astr0[:].rearrange("p (h c g) -> p c h g",
                                             h=H, g=G1)[:, cc, :, :]
                    nc.tensor.matmul(pg0[:, 16 * cc:16 * cc + 16],
                                     h1row[:, cc * FD:(cc + 1) * FD], rhs)
                aggT0 = big.tile([128, 2 * G1 * NC1], BF, tag="aggT0")
                cp(aggT0[:], pg0[:], "scalar")
                g0t = big.tile([128, 2 * B], BF, tag="g0t")
                pj0 = ppagg.tile([128, 2 * B], F32, tag="pagg", name="pj0")
                for h in range(H):
                    rhs = aggT0[:].rearrange("p (c h g) -> p h c g",
                                             h=H, g=G1)[:, h, :, :]
                    nc.tensor.matmul(pj0[:, B * h:B * (h + 1)],
                                     S["W0c"][:, 128 * h:128 * (h + 1)], rhs)
                cp(g0t[:], pj0[:], "scalar")
                for h in range(H):
                    pes = ppagg.tile([128, 480], F32, tag="pagg", name="pesL")
                    for hp in range(H):
                        nc.tensor.matmul(pes[:1, :B],
                                         S["u1"][:, 2 * hp + h:2 * hp + h + 1],
                                         g0t[:, B * hp:B * (hp + 1)],
                                         start=(hp == 0), stop=(hp == 1))
                    eh = big.tile([1, B], BF, tag=f"esLh{h}")
                    cp(eh[:], pes[:1, :B], "scalar")
                    esLh.append(eh)

            x1t = big.tile([128, M1], BF, tag="x1t")

            # ================= j1 stream =================
            g1t = big.tile([128, 2 * M1], BF, tag="g1t")          # (h, m)
            g1row = big.tile([T1, NC1 * 2 * FD], BF, tag="g1row")  # (t,(hp f))
            penL = ppenL.tile([T1, 2 * NC1], F32, tag="penL", name="penL")
            nc.tensor.matmul(penL[:], maskE10b(0), S["zrow"][0:1, :2 * NC1],
                             start=True, stop=False, skip_group_check=True)
            g1row_done = [0]
            copy_rr = [0]

            def rr(seq=("vector", "scalar")):
                copy_rr[0] += 1
                return seq[copy_rr[0] % len(seq)]

            def enL_for_t(t):
                # en_L1 accumulation for finished 80-row window t
                for h in range(H):
                    for hp in range(H):
                        nc.tensor.matmul(
                            penL[:, NC1 * h + t: NC1 * h + t + 1],
                            g1t[:, M1 * hp + T1 * t: M1 * hp + T1 * (t + 1)],
                            S["v1"][:, 2 * hp + h:2 * hp + h + 1],
                            start=False, stop=False,
                            skip_group_check=True)

            def g1row_for_t(t):
                ptg = pt_tile("ptg1")
                for hp in range(H):
                    nc.tensor.transpose(
                        ptg[:T1, FD * hp:FD * (hp + 1)],
                        g1t[:, M1 * hp + T1 * t: M1 * hp + T1 * (t + 1)],
                        identb[:])
                cp(g1row[:, 2 * FD * t:2 * FD * (t + 1)], ptg[:T1, :2 * FD],
                   "vector")
                enL_for_t(t)

            def stage1(b):
                hb, bn, c0 = hbs[b], BLOCKS[b][0], c0s[b]
                pen = ppen.tile([T2, 2 * bn], F32, tag="pen", name="penj1")
                nc.tensor.matmul(pen[:], maskE25b(0), S["zrow"][0:1, :2 * bn],
                                 start=True, stop=False, skip_group_check=True)
                xts = []
                for sub in range(0, bn, 4):
                    sn = min(4, bn - sub)
                    pt3 = pt_tile("pt3")
                    for j in range(sn):
                        cl = sub + j
                        nc.tensor.transpose(pt3[:, j * 126:j * 126 + T2],
                                            hb[:, cl * FD:(cl + 1) * FD],
                                            identb[:T2, :T2])
                    xs = x2tp.tile([128, sn * T2], BF, tag="x2t", name="xs")
                    src = pt3[:, :sn * 126].rearrange(
                        "p (j c) -> p j c", c=126)[:, :, :T2]
                    cp(xs[:].rearrange("p (j c) -> p j c", c=T2), src,
                       rr(("vector", "vector", "scalar")))
                    xts.append((sub, sn, xs))
                for (sub, sn, xs) in xts:
                    for j in range(sn):
                        cl = sub + j
                        for h in range(H):
                            nc.tensor.matmul(
                                pen[:, bn * h + cl: bn * h + cl + 1],
                                xs[:, j * T2:(j + 1) * T2],
                                S["v0"][:, h:h + 1],
                                start=False, stop=False,
                                skip_group_check=True)
                return pen

            def stage2(b, pen):
                hb, bn, c0 = hbs[b], BLOCKS[b][0], c0s[b]
                # add self-logit broadcast into pen (cols h-major)
                for g in range(G2):
                    for h in range(H):
                        nc.tensor.matmul(
                            pen[:, bn * h:bn * (h + 1)],
                            maskE25b(g),
                            es1h[h][:].rearrange("o (c g) -> o c g",
                                                 g=G2)[:, c0:c0 + bn, g],
                            start=False, stop=(g == G2 - 1 and h == H - 1),
                            skip_group_check=True)
                astr = softmax_alpha(T2, G2, 2 * bn, pen[:], mask25,
                                     maskE25f, "j1")
                # agg into [128 f, (cl, h, g)] then project this m-window
                pagg = ppagg.tile([128, 10 * bn], F32, tag="pagg",
                                  name="paggj1")
                for cl in range(bn):
                    rhs = astr[:].rearrange("p (h c g) -> p c h g",
                                            h=H, g=G2)[:, cl, :, :]
                    nc.tensor.matmul(pagg[:, 10 * cl:10 * cl + 10],
                                     hb[:, cl * FD:(cl + 1) * FD], rhs)
                aggT = agt.tile([128, 10 * bn], BF, tag="aggT", name="aggT")
                cp(aggT[:], pagg[:], "scalar")
                pj = ppagg.tile([128, 2 * G2 * bn], F32, tag="pagg",
                                name="pjj1")
                for h in range(H):
                    rhs = aggT[:].rearrange("p (c h g) -> p h c g",
                                            h=H, g=G2)[:, h, :, :]
                    nc.tensor.matmul(pj[:, G2 * bn * h:G2 * bn * (h + 1)],
                                     S["W0c"][:, 128 * h:128 * (h + 1)], rhs)
                m0, mn = G2 * c0, G2 * bn
                for h in range(H):
                    cp(g1t[:, M1 * h + m0: M1 * h + m0 + mn],
                       pj[:, mn * h:mn * (h + 1)], ("scalar", "vector")[h])
                while T1 * (g1row_done[0] + 1) <= m0 + mn:
                    g1row_for_t(g1row_done[0])
                    g1row_done[0] += 1

            # software pipeline over blocks; keep DMAs ~3 blocks ahead
            pens = {}
            pens[0] = stage1(0)
            j0a()
            next_dma = 3
            for b in range(1, len(BLOCKS)):
                pens[b] = stage1(b)
                if next_dma < len(BLOCKS):
                    issue_hb(next_dma)
                    next_dma += 1
                stage2(b - 1, pens.pop(b - 1))
                if b == 1:
                    j0b()
            stage2(len(BLOCKS) - 1, pens.pop(len(BLOCKS) - 1))

            # ================= L1 =================
            # finish penL: add esL broadcast
            for g in range(G1):
                for h in range(H):
                    nc.tensor.matmul(
                        penL[:, NC1 * h:NC1 * (h + 1)],
                        maskE10b(g),
                        esLh[h][:].rearrange("o (c g) -> o c g",
                                             g=G1)[:, :, g],
                        start=False, stop=(g == G1 - 1 and h == H - 1),
                        skip_group_check=True)
            astrL = softmax_alpha(T1, G1, 2 * NC1, penL[:], mask10,
                                  maskE10f, "L1")
            # aggT2 [128 (fp-slab d), (fp, t, h, g)]
            aggT2 = big.tile([128, 2 * 256], BF, tag="aggT2")
            for fp in range(2):
                pg = ppagg.tile([128, 256], F32, tag="pagg", name="pgL")
                for t in range(NC1):
                    rhs = astrL[:].rearrange("p (h c g) -> p c h g",
                                             h=H, g=G1)[:, t, :, :]
                    nc.tensor.matmul(pg[:, 16 * t:16 * t + 16],
                                     g1row[:, (2 * t + fp) * FD:
                                              (2 * t + fp + 1) * FD], rhs)
                cp(aggT2[:, 256 * fp:256 * (fp + 1)], pg[:],
                   ("vector", "scalar")[fp])

            # L1 projection -> ggt [128 d, (h, m0)]
            ggt = big.tile([128, 2 * B], BF, tag="ggt")
            W1cs = (S["W1ca"], S["W1cb"])
            pjL = ppagg.tile([128, 2 * B], F32, tag="pagg", name="pjL")
            for h in range(H):
                for fp in range(2):
                    rhs = aggT2[:, 256 * fp:256 * (fp + 1)].rearrange(
                        "p (t h g) -> p h t g", h=H, g=G1)[:, h, :, :]
                    nc.tensor.matmul(pjL[:, B * h:B * (h + 1)],
                                     W1cs[fp][:, 128 * h:128 * (h + 1)], rhs,
                                     start=(fp == 0), stop=(fp == 1))
            cp(ggt[:], pjL[:], "vector")

            # fc + final transpose
            pfc = ppagg.tile([128, B], F32, tag="pagg", name="pfc")
            fcs = (S["fcWa"], S["fcWb"])
            for hp in range(H):
                nc.tensor.matmul(pfc[:], fcs[hp][:],
                                 ggt[:, B * hp:B * (hp + 1)],
                                 start=(hp == 0), stop=(hp == 1))
            outT = big.tile([128, B], BF, tag="outT")
            cp(outT[:], pfc[:], "scalar")
            ptf = pt_tile("ptf")
            nc.tensor.transpose(ptf[:, :B], outT[:], identb[:])
            outn = big.tile([B, OUT], F32, tag="outn")
            cp(outn[:], ptf[:, :B], "vector")
            nc.sync.dma_start(y[:], outn[:])

    if split_waits:
        _split_multi_waits(nc)
    return nc


_PROG = None


def kernel(**inputs):
    global _PROG
    _install_drain_patch()
    P = host_params(inputs["W0"], inputs["a_self0"], inputs["a_neigh0"],
                    inputs["W1"], inputs["a_self1"], inputs["a_neigh1"],
                    inputs["fc_W"])
    if _PROG is None:
        _PROG = build_program()
    nc = _PROG
    h0 = np.asarray(inputs["h0"], np.float32)
    h1 = np.asarray(inputs["h1"], np.float32)
    h2 = np.asarray(inputs["h2"], np.float32)
    in_maps = []
    for c in range(NCORES):
        m = {"h0p": np.ascontiguousarray(
                 h0[B * c:B * (c + 1)].astype(NPBF)),
             "h1p": pack_h(h1[M1 * c:M1 * (c + 1)], T1),
             "h2p": pack_h(h2[M2 * c:M2 * (c + 1)], T2)}
        m.update(P)
        in_maps.append(m)
    core_ids = list(range(NCORES))
    last = None
    for _attempt in range(3):
        try:
            res = run_bass_kernel_spmd(nc, in_maps, core_ids)
            out = np.concatenate([np.asarray(res.results[c]["y"])
                                  for c in core_ids], axis=0)
            return out.astype(np.float32)
        except Exception as e:   # transient device-unrecoverable happens
            last = e
    raise last
